# revision 20
# baseline (speedup 1.0000x reference)
"""ConvAttention kernel for 8x TRN2 NeuronCores.

Model (per batch item b):
    q/k/v = grouped_conv1d(x_b, w, b, groups=8)        # [E, T] -> [E, T]
    S     = (q^T k) / sqrt(E)                          # [T, T]
    P     = softmax(S, axis=-1)
    y     = (P @ v^T) @ w_fc^T + b_fc                  # [T, E]

Sharding: pure data-parallel over batch B=8 -> 8 cores, weights replicated.

Per-core algorithm (no transposes, scores never leave the chip):
  * x lives fully resident in SBUF as bf16 (host-padded halo); conv
    projections as block-diagonal [128,128] bf16 matmuls per tap, output in
    "ET" layout (channels on partitions) f32r -- what the scores matmul wants.
  * phase-1 structure: PE-warmup dummies (p-state ramp) -> k-pass -> v-pass
    -> q-pass with vw matmuls interleaved -> attention.  Everything the
    attention needs (k, q, vw) is finished when it starts, so the PE never
    stalls at the phase boundary.
  * fc is pushed in front of attention by associativity:
        y = P_norm @ (v_c @ w_fc^T + 1*beff)   with beff = w_fc@bv + b_fc
    (v's conv bias bv commutes through the softmax-normalized P).
  * scores are computed TRANSPOSED (S^T tiles, lhsT=k-tile, rhs=q-chunk) so
    that after exp the tiles are directly the stationary operand of attn@v.
  * softmax without max-subtraction (scores ~ N(0,1), exp is safe in fp32);
    row sums come for free from a ones-column appended to vw -> normalization
    is a per-partition reciprocal+scale on the final [128, 256] tiles.
  * scores matmuls in fp32r (full PE speed at N>=256); q/k tiles are
    *produced* as float32r by the bias-add, as walrus requires.
  * attention inner loop: per si-pair, 4 S^T matmuls -> one N=1024 exp
    (shifted by -3, output fp8e4) -> 4 fp8 DoubleRow attn@v matmuls (each
    contracts the full si-pair, K=256, at 2 MACs/cell/cycle) accumulating
    into 4 per-t-subtile PSUM banks, software-pipelined (S^T of pair p+1
    before attn@v of pair p).
  * fp8 error budget (simulated): P,vw in e4m3 + bf16 x/conv ->
    rel err ~1.6e-2 < 2e-2; scores stay fp32r (q/k fp8 would push it over).
"""

import contextlib

import ml_dtypes
import numpy as np

import concourse.bacc as bacc
import concourse.mybir as mybir
import concourse.tile as tile
from concourse.bass_utils import run_bass_kernel_spmd

dt = mybir.dt
AF = mybir.ActivationFunctionType
DR = mybir.MatmulPerfMode.DoubleRow
EXP_SHIFT = 3.0  # softmax shift: keeps exp() in fp8e4 range (max ~e^3.2 << 240)

B, E, T, H, KW = 8, 256, 4096, 8, 3
NCORES = 8
P = 128                  # partitions / half of E
TCH = 512                # t-chunk width
NCH = T // TCH           # 8 chunks
NST = T // P             # 32 s-tiles
NSUB = TCH // P          # 4 t-subtiles per chunk
NPAIR = NST // 2         # 16 si-pairs
EA = E + 2               # vw width incl. ones column (padded even for fp32r)
TP = T + 2               # padded x width (halo)
WCOLS = 2 * KW * P       # one projection's weight pack: 768 bf16 columns
XWK = 0                  # flat-x column offsets: wk pack leads
XBK = XWK + WCOLS        # bk (2 bf16 cols)
XBQ = XBK + 2            # bq (2 bf16 cols)
XH0 = XBQ + 2            # x half 0 (halo-padded)
XH1 = XH0 + TP           # x half 1
XWV = XH1 + TP           # wv pack
XWQ = XWV + WCOLS        # wq pack
XCOLS = XWQ + WCOLS      # total flat-x row width
NDUMMY = 8              # PE warm-up matmuls while first DMAs land

TRACE = False
LAST = {}

_MODULE = None


def _build(tc, io):
    nc = tc.nc
    f32 = dt.float32
    f32r = dt.float32r
    bf16 = dt.bfloat16
    f8 = dt.float8e4
    x_d, wf_d, be_d, oc_d, y_d = io

    with contextlib.ExitStack() as ctx:
        const_p = ctx.enter_context(tc.tile_pool(name="const", bufs=1))
        big_p = ctx.enter_context(tc.tile_pool(name="big", bufs=1))
        pt_p = ctx.enter_context(tc.tile_pool(name="ptp", bufs=6))
        out_p = ctx.enter_context(tc.tile_pool(name="outp", bufs=4))

        # ---- PE warm-up: scratch memsets (first gpsimd work), dummy matmuls
        # ramp the Tensor engine p-state while the first DMAs are in flight.
        scr_w = const_p.tile([P, P], bf16, tag="scrw", name="scr_w")
        nc.gpsimd.memset(scr_w[:], 0)
        scr_x = const_p.tile([P, TCH], bf16, tag="scrx", name="scr_x")
        nc.gpsimd.memset(scr_x[:], 0)

        # ---- DMA plan.  Facts: HWDGE descriptors cap at ~8KB/row (bigger
        # rows split and halve throughput), the sync ring starts ~8.6us,
        # the scalar ring ~11.3us, the gpsimd software-DGE ring ~14us and
        # slow.  x is one flat row [h0 | h1 | wk | bk | bq | wv | wq]:
        # (1) wk+biases (1.5KB rows) lead the sync ring -> land ~9.6us,
        # (2) h0 follows in two column halves, streaming just ahead of the
        #     k-pass,
        # (3) h1 halves ride the scalar ring,
        # (4) wv|wq close the sync ring, landing before the v-pass.
        x_sb = big_p.tile([P, XCOLS], bf16, tag="x", name="x_sb")
        TH = TP // 2 + 1
        c1 = XH0 + 2 * TCH + 2
        nc.sync.dma_start(out=x_sb[:, 0:c1], in_=x_d[:, 0:c1])
        for pc in range(3):
            c2 = min(c1 + 2 * TCH, XH1)
            nc.sync.dma_start(out=x_sb[:, c1:c2], in_=x_d[:, c1:c2])
            c1 = c2
        nc.scalar.dma_start(out=x_sb[:, XH1 : XH1 + TH], in_=x_d[:, XH1 : XH1 + TH])
        nc.scalar.dma_start(out=x_sb[:, XH1 + TH : XWV], in_=x_d[:, XH1 + TH : XWV])
        nc.sync.dma_start(out=x_sb[:, XWV:XCOLS], in_=x_d[:, XWV:XCOLS])

        def w_slice(pi, h, kk):
            base = {1: XWK, 2: XWV, 0: XWQ}[pi]
            c0 = base + (h * KW + kk) * P
            return x_sb[:, c0 : c0 + P]

        # biases ride packed in x as bf16; widen to f32 on-chip (DVE
        # tensor_scalar requires an f32 scalar operand)
        bkq_sb = const_p.tile([P, 4], f32, tag="bkq", name="bkq_sb")
        nc.vector.tensor_copy(bkq_sb[:], x_sb[:, XBK : XBK + 4])
        sh_sb = const_p.tile([P, 1], f32, tag="sh", name="shift_sb")
        nc.gpsimd.memset(sh_sb[:], -EXP_SHIFT)
        wf_sb = []
        for h in range(2):
            wft = const_p.tile([P, E], f32r, tag=f"wf{h}", name=f"wf{h}")
            nc.gpsimd.dma_start(out=wft[:], in_=wf_d[h])
            wf_sb.append(wft)
        be_sb = const_p.tile([P, E], f32, tag="be", name="be_sb")
        nc.gpsimd.dma_start(out=be_sb[:], in_=be_d[:])

        # ---------------- resident tensors ----------------
        k_sb = []
        q_sb = []
        v_sb = []
        for h in range(2):
            k_sb.append(big_p.tile([P, T], f32r, tag=f"k{h}", name=f"k{h}"))
            q_sb.append(big_p.tile([P, T], f32r, tag=f"q{h}", name=f"q{h}"))
            v_sb.append(big_p.tile([P, T], f32r, tag=f"v{h}", name=f"v{h}"))
        vw_sb = big_p.tile([P, NST, EA], f8, tag="vw", name="vw_sb")
        nc.gpsimd.dma_start(
            out=vw_sb[:, :, E:EA], in_=oc_d[:].rearrange("p (n o) -> p n o", o=2)
        )

        # ---------------- phase 1: q, k, v -> vw' ----------------
        with (
            tc.tile_pool(name="ps_cv", bufs=4, space="PSUM") as ps_cv,
            tc.tile_pool(name="ps_vw", bufs=2, space="PSUM") as ps_vw_p,
        ):
            # dummy matmuls: ramp PE while wk/x DMAs land (results
            # discarded).  Grouped 4 per PSUM tile with a 1-column release
            # read -- too many unread accumulation groups stall the PE.
            scr_rd = const_p.tile([P, NDUMMY // 4], f32, tag="scrr", name="scr_rd")
            for b in range(NDUMMY // 4):
                ps_scr = ps_cv.tile([P, TCH], f32, tag="cv", name="ps_scr")
                for _ in range(4):
                    nc.tensor.matmul(
                        ps_scr[:], scr_w[:], scr_x[:], start=True, stop=True
                    )
                nc.vector.tensor_copy(scr_rd[:, b : b + 1], ps_scr[:, 0:1])

            def conv_pass(pi, emit):
                for h in range(2):
                    for j in range(NCH):
                        ps = ps_cv.tile([P, TCH], f32, tag="cv", name="ps_cv")
                        for kk in range(KW):
                            nc.tensor.matmul(
                                ps[:],
                                w_slice(pi, h, kk),
                                x_sb[:, XH0 + h * TP + j * TCH + kk : XH0 + h * TP + j * TCH + kk + TCH],
                                start=(kk == 0),
                                stop=(kk == KW - 1),
                            )
                        emit(h, j, ps)

            tsl_of = lambda j: slice(j * TCH, (j + 1) * TCH)
            conv_pass(
                1,
                lambda h, j, ps: nc.vector.tensor_scalar_add(
                    k_sb[h][:, tsl_of(j)], ps[:], bkq_sb[:, h : h + 1]
                ),
            )
            conv_pass(
                2, lambda h, j, ps: nc.vector.tensor_copy(v_sb[h][:, tsl_of(j)], ps[:])
            )

            # q-pass with vw matmuls interleaved at tap granularity: each
            # 512-col q-conv matmul hides the next vw LDWEIGHTS, so the
            # short (256-col) vw matmuls run back-to-back at full rate
            for h in range(2):
                for j in range(NCH):
                    jh = h * NCH + j
                    ps = ps_cv.tile([P, TCH], f32, tag="cv", name="ps_cv")
                    ps_v = ps_vw_p.tile([P, 2, E], f32, tag="vw", name="ps_vw")
                    vw_mms = []
                    for u in range(2):
                        tsl = slice((2 * jh + u) * P, (2 * jh + u + 1) * P)
                        vw_mms.append((ps_v[:, u, :], v_sb[0][:, tsl], wf_sb[0], True, False))
                        vw_mms.append((ps_v[:, u, :], v_sb[1][:, tsl], wf_sb[1], False, True))
                    for kk in range(KW):
                        nc.tensor.matmul(
                            ps[:],
                            w_slice(0, h, kk),
                            x_sb[:, XH0 + h * TP + j * TCH + kk : XH0 + h * TP + j * TCH + kk + TCH],
                            start=(kk == 0),
                            stop=(kk == KW - 1),
                        )
                        o, l, r, st, sp = vw_mms[kk]
                        nc.tensor.matmul(o, l[:], r[:], start=st, stop=sp)
                    o, l, r, st, sp = vw_mms[3]
                    nc.tensor.matmul(o, l[:], r[:], start=st, stop=sp)
                    nc.vector.tensor_scalar_add(
                        q_sb[h][:, tsl_of(j)], ps[:], bkq_sb[:, 2 + h : 3 + h]
                    )
                    nc.scalar.activation(
                        vw_sb[:, 2 * jh : 2 * jh + 2, 0:E], ps_v[:], AF.Copy
                    )

        # ---------------- phase 2: attention ----------------
        with (
            tc.tile_pool(name="ps_st", bufs=2, space="PSUM") as ps_st,
            tc.tile_pool(name="ps_u", bufs=1, space="PSUM") as ps_u,
        ):
            def st_pair(j, p):
                """S^T matmuls + one wide exp for si = 2p, 2p+1 vs chunk j."""
                ps = ps_st.tile([P, 2, TCH], f32, tag="st", name="ps_st")
                pt = pt_p.tile([P, 2, TCH], f8, tag="pt", name="pt")
                csl = slice(j * TCH, (j + 1) * TCH)
                for d in range(2):
                    ssl = slice((2 * p + d) * P, (2 * p + d + 1) * P)
                    nc.tensor.matmul(
                        ps[:, d, :], k_sb[0][:, ssl], q_sb[0][:, csl],
                        start=True, stop=False,
                    )
                    nc.tensor.matmul(
                        ps[:, d, :], k_sb[1][:, ssl], q_sb[1][:, csl],
                        start=False, stop=True,
                    )
                nc.scalar.activation(pt[:], ps[:], AF.Exp, bias=sh_sb[:])
                return pt

            def u_pair(p, pt, ups):
                """fp8 DoubleRow attn@v for si pair (2p, 2p+1): one matmul
                per t-subtile contracts both s-tiles (K=256) at 2x rate."""
                for ti in range(NSUB):
                    nc.tensor.matmul(
                        ups[ti][:],
                        pt[:, :, ti * P : (ti + 1) * P],
                        vw_sb[:, 2 * p : 2 * p + 2, :],
                        start=(p == 0),
                        stop=(p == NPAIR - 1),
                        perf_mode=DR,
                    )

            def drain(j, ups):
                """normalize + bias + store chunk j's four t-subtiles;
                y-store DMA issues alternate sync/scalar queues so the final
                chunk's stores overlap their own issue latency."""
                for ti in range(NSUB):
                    t0 = j * TCH + ti * P
                    rec = out_p.tile([P, 1], f32, tag="rec", name="rec")
                    nc.vector.reciprocal(rec[:], ups[ti][:, E : E + 1])
                    yt = out_p.tile([P, E], f32, tag="yt", name="yt")
                    nc.vector.scalar_tensor_tensor(
                        yt[:],
                        ups[ti][:, 0:E],
                        rec[:],
                        be_sb[:],
                        op0=mybir.AluOpType.mult,
                        op1=mybir.AluOpType.add,
                    )
                    eng = nc.scalar if (j == NCH - 1 and ti % 2 == 1) else nc.sync
                    eng.dma_start(out=y_d[t0 : t0 + P, :], in_=yt[:])

            # flat software pipeline with lag 2 ACROSS chunk boundaries:
            # attn@v for slot i runs after S^T of slot i+2, so each exp has a
            # full extra slot to finish, and the PE never drains between
            # chunks (chunk j+1's S^T matmuls overlap chunk j's tail attn@v
            # and its vector drain).
            LAG = 4
            slots = [(j, p) for j in range(NCH) for p in range(NPAIR)]
            ups_by_j = {}
            pts = {}

            def consume(i):
                j2, p2 = slots[i]
                if p2 == 0:
                    # allocate chunk j2's accumulators at first use; the
                    # prior chunk with these tags has fully drained by now
                    ups_by_j[j2] = [
                        ps_u.tile([P, EA], f32, tag=f"u{ti}", name=f"ups{ti}")
                        for ti in range(NSUB)
                    ]
                u_pair(p2, pts.pop(i), ups_by_j[j2])
                if p2 == NPAIR - 1:
                    drain(j2, ups_by_j.pop(j2))

            for i, (j, p) in enumerate(slots):
                pts[i] = st_pair(j, p)
                if i >= LAG:
                    consume(i - LAG)
            for i in range(len(slots) - LAG, len(slots)):
                consume(i)


def build_module():
    """Build + compile the Bass module (cached)."""
    global _MODULE
    if _MODULE is not None:
        return _MODULE
    nc = bacc.Bacc(
        "TRN2",
        target_bir_lowering=False,
        debug=False,
        enable_asserts=False,
        num_devices=NCORES,
    )
    f32 = dt.float32
    f32r = dt.float32r
    bf16 = dt.bfloat16
    x_d = nc.dram_tensor("x", [P, XCOLS], bf16, kind="ExternalInput").ap()
    wf_d = nc.dram_tensor("wfcT", [2, P, E], f32r, kind="ExternalInput").ap()
    be_d = nc.dram_tensor("beff", [P, E], f32, kind="ExternalInput").ap()
    oc_d = nc.dram_tensor("onescol", [P, NST * 2], dt.float8e4, kind="ExternalInput").ap()
    y_d = nc.dram_tensor("y", [T, E], f32, kind="ExternalOutput").ap()

    with tile.TileContext(nc) as tc:
        _build(tc, (x_d, wf_d, be_d, oc_d, y_d))
    nc.compile()
    _MODULE = nc
    return nc


def _marshal(wq, bq, wk, bk, wv, bv, w_fc, b_fc):
    """Host-side input prep (weights only -- all tiny)."""
    scale = np.float32(1.0 / np.sqrt(E))

    def blockdiag(w):
        # w: [E, E//H, KW] grouped conv weight ->
        # out[h, in_local, kk, out_local] block-diagonal per half.
        out = np.zeros((2, P, KW, P), np.float32)
        gs = E // H  # 32
        for h in range(2):
            for g in range(4):
                grp = 4 * h + g
                blk = w[gs * grp : gs * (grp + 1), :, :]  # [out c', in i, kk]
                for kk in range(KW):
                    out[h, gs * g : gs * (g + 1), kk, gs * g : gs * (g + 1)] = blk[
                        :, :, kk
                    ].T
        return out

    wqb = blockdiag(wq) * scale
    wkb = blockdiag(wk)
    wvb = blockdiag(wv)
    bq2 = np.ascontiguousarray((bq * scale).reshape(2, P).T)
    bk2 = np.ascontiguousarray(bk.reshape(2, P).T)
    wfcT = np.ascontiguousarray(w_fc.T.reshape(2, P, E))
    beff = np.ascontiguousarray(
        np.broadcast_to((w_fc @ bv + b_fc).reshape(1, E), (P, E))
    )
    bf = ml_dtypes.bfloat16
    # weight packs as [P, 768] bf16 rows: (h, kk, out) flattened per in-chan
    def wpack(w):
        return np.ascontiguousarray(
            w.transpose(1, 0, 2, 3).reshape(P, WCOLS).astype(bf)
        )
    return {
        "wkp": wpack(wkb),
        "wvp": wpack(wvb),
        "wqp": wpack(wqb),
        "bq2": bq2.astype(bf),
        "bk2": bk2.astype(bf),
        "wfcT": wfcT,
        "beff": beff,
        "onescol": np.ones((P, NST * 2), ml_dtypes.float8_e4m3),
    }


def kernel(x, wq, bq, wk, bk, wv, bv, w_fc, b_fc, num_heads):
    x = np.asarray(x, np.float32)
    consts = _marshal(
        np.asarray(wq, np.float32),
        np.asarray(bq, np.float32),
        np.asarray(wk, np.float32),
        np.asarray(bk, np.float32),
        np.asarray(wv, np.float32),
        np.asarray(bv, np.float32),
        np.asarray(w_fc, np.float32),
        np.asarray(b_fc, np.float32),
    )
    nc = build_module()
    # per-core flat x row: [h0 | h1 | wk | bk | bq | wv | wq] in bf16,
    # zero halo cols at both ends of each half
    xp = np.zeros((B, P, XCOLS), ml_dtypes.bfloat16)
    xb = x.astype(ml_dtypes.bfloat16)
    for b in range(B):
        for h in range(2):
            c0 = XH0 + h * TP + 1
            xp[b, :, c0 : c0 + T] = xb[b, h * P : (h + 1) * P, :]
        xp[b, :, XWK:XBK] = consts["wkp"]
        xp[b, :, XBK : XBK + 2] = consts["bk2"]
        xp[b, :, XBQ : XBQ + 2] = consts["bq2"]
        xp[b, :, XWV:XWQ] = consts["wvp"]
        xp[b, :, XWQ:XCOLS] = consts["wqp"]
    drop = ("wkp", "wvp", "wqp", "bq2", "bk2")
    wconsts = {k: v for k, v in consts.items() if k not in drop}
    in_maps = [{"x": np.ascontiguousarray(xp[b]), **wconsts} for b in range(B)]
    res = run_bass_kernel_spmd(nc, in_maps, core_ids=list(range(NCORES)), trace=TRACE)
    LAST["exec_time_ns"] = res.exec_time_ns
    LAST["mean_exec_time_ns"] = res.mean_exec_time_ns
    LAST["results"] = res
    out = np.stack([res.results[b]["y"] for b in range(B)], axis=0)
    return out


# revision 21
# speedup vs baseline: 1.0292x; 1.0292x over previous
"""ConvAttention kernel for 8x TRN2 NeuronCores.

Model (per batch item b):
    q/k/v = grouped_conv1d(x_b, w, b, groups=8)        # [E, T] -> [E, T]
    S     = (q^T k) / sqrt(E)                          # [T, T]
    P     = softmax(S, axis=-1)
    y     = (P @ v^T) @ w_fc^T + b_fc                  # [T, E]

Sharding: pure data-parallel over batch B=8 -> 8 cores, weights replicated.

Per-core algorithm (no transposes, scores never leave the chip):
  * x lives fully resident in SBUF as bf16 (host-padded halo); conv
    projections as block-diagonal [128,128] bf16 matmuls per tap, output in
    "ET" layout (channels on partitions) f32r -- what the scores matmul wants.
  * phase-1 structure: PE-warmup dummies (p-state ramp) -> k-pass -> v-pass
    -> q-pass with vw matmuls interleaved -> attention.  Everything the
    attention needs (k, q, vw) is finished when it starts, so the PE never
    stalls at the phase boundary.
  * fc is pushed in front of attention by associativity:
        y = P_norm @ (v_c @ w_fc^T + 1*beff)   with beff = w_fc@bv + b_fc
    (v's conv bias bv commutes through the softmax-normalized P).
  * scores are computed TRANSPOSED (S^T tiles, lhsT=k-tile, rhs=q-chunk) so
    that after exp the tiles are directly the stationary operand of attn@v.
  * softmax without max-subtraction (scores ~ N(0,1), exp is safe in fp32);
    row sums come for free from a ones-column appended to vw -> normalization
    is a per-partition reciprocal+scale on the final [128, 256] tiles.
  * scores matmuls in fp32r (full PE speed at N>=256); q/k tiles are
    *produced* as float32r by the bias-add, as walrus requires.
  * attention inner loop: per si-pair, 4 S^T matmuls -> one N=1024 exp
    (shifted by -3, output fp8e4) -> 4 fp8 DoubleRow attn@v matmuls (each
    contracts the full si-pair, K=256, at 2 MACs/cell/cycle) accumulating
    into 4 per-t-subtile PSUM banks, software-pipelined (S^T of pair p+1
    before attn@v of pair p).
  * fp8 error budget (simulated): P,vw in e4m3 + bf16 x/conv ->
    rel err ~1.6e-2 < 2e-2; scores stay fp32r (q/k fp8 would push it over).
"""

import contextlib

import ml_dtypes
import numpy as np

import concourse.bacc as bacc
import concourse.mybir as mybir
import concourse.tile as tile
from concourse.bass_utils import run_bass_kernel_spmd

dt = mybir.dt
AF = mybir.ActivationFunctionType
DR = mybir.MatmulPerfMode.DoubleRow
EXP_SHIFT = 3.0  # softmax shift: keeps exp() in fp8e4 range (max ~e^3.2 << 240)

B, E, T, H, KW = 8, 256, 4096, 8, 3
NCORES = 8
P = 128                  # partitions / half of E
TCH = 512                # t-chunk width
NCH = T // TCH           # 8 chunks
NST = T // P             # 32 s-tiles
NSUB = TCH // P          # 4 t-subtiles per chunk
NPAIR = NST // 2         # 16 si-pairs
EA = E + 2               # vw width incl. ones column (padded even for fp32r)
TP = T + 2               # padded x width (halo)
WCOLS = 2 * KW * P       # one projection's weight pack: 768 bf16 columns
XWK = 0                  # flat-x column offsets: wk pack leads
XBK = XWK + WCOLS        # bk (2 bf16 cols)
XBQ = XBK + 2            # bq (2 bf16 cols)
XH0 = XBQ + 2            # x half 0 (halo-padded)
XH1 = XH0 + TP           # x half 1
XWV = XH1 + TP           # wv pack
XWQ = XWV + WCOLS        # wq pack
XCOLS = XWQ + WCOLS      # total flat-x row width
NDUMMY = 12              # PE warm-up matmuls while first DMAs land

TRACE = False
LAST = {}

_MODULE = None


def _build(tc, io):
    nc = tc.nc
    f32 = dt.float32
    f32r = dt.float32r
    bf16 = dt.bfloat16
    f8 = dt.float8e4
    x_d, wf_d, be_d, oc_d, y_d = io

    with contextlib.ExitStack() as ctx:
        const_p = ctx.enter_context(tc.tile_pool(name="const", bufs=1))
        big_p = ctx.enter_context(tc.tile_pool(name="big", bufs=1))
        pt_p = ctx.enter_context(tc.tile_pool(name="ptp", bufs=6))
        out_p = ctx.enter_context(tc.tile_pool(name="outp", bufs=4))

        # ---- PE warm-up: scratch memsets (first gpsimd work), dummy matmuls
        # ramp the Tensor engine p-state while the first DMAs are in flight.
        scr_w = const_p.tile([P, P], bf16, tag="scrw", name="scr_w")
        nc.gpsimd.memset(scr_w[:], 0)
        scr_x = const_p.tile([P, TCH], bf16, tag="scrx", name="scr_x")
        nc.gpsimd.memset(scr_x[:], 0)

        # ---- DMA plan.  Facts: HWDGE descriptors cap at ~8KB/row (bigger
        # rows split and halve throughput), the sync ring starts ~8.6us,
        # the scalar ring ~11.3us, the gpsimd software-DGE ring ~14us and
        # slow.  x is one flat row [h0 | h1 | wk | bk | bq | wv | wq]:
        # (1) wk+biases (1.5KB rows) lead the sync ring -> land ~9.6us,
        # (2) h0 follows in two column halves, streaming just ahead of the
        #     k-pass,
        # (3) h1 halves ride the scalar ring,
        # (4) wv|wq close the sync ring, landing before the v-pass.
        x_sb = big_p.tile([P, XCOLS], bf16, tag="x", name="x_sb")
        TH = TP // 2 + 1
        nc.sync.dma_start(out=x_sb[:, 0 : XH0 + TH], in_=x_d[:, 0 : XH0 + TH])
        nc.sync.dma_start(out=x_sb[:, XH0 + TH : XH1], in_=x_d[:, XH0 + TH : XH1])
        nc.scalar.dma_start(out=x_sb[:, XH1 : XH1 + TH], in_=x_d[:, XH1 : XH1 + TH])
        nc.scalar.dma_start(out=x_sb[:, XH1 + TH : XWV], in_=x_d[:, XH1 + TH : XWV])
        nc.sync.dma_start(out=x_sb[:, XWV:XCOLS], in_=x_d[:, XWV:XCOLS])

        def w_slice(pi, h, kk):
            base = {1: XWK, 2: XWV, 0: XWQ}[pi]
            c0 = base + (h * KW + kk) * P
            return x_sb[:, c0 : c0 + P]

        # biases ride packed in x as bf16; widen to f32 on-chip (DVE
        # tensor_scalar requires an f32 scalar operand)
        bkq_sb = const_p.tile([P, 4], f32, tag="bkq", name="bkq_sb")
        nc.vector.tensor_copy(bkq_sb[:], x_sb[:, XBK : XBK + 4])
        sh_sb = const_p.tile([P, 1], f32, tag="sh", name="shift_sb")
        nc.gpsimd.memset(sh_sb[:], -EXP_SHIFT)
        wf_sb = []
        for h in range(2):
            wft = const_p.tile([P, E], f32r, tag=f"wf{h}", name=f"wf{h}")
            nc.gpsimd.dma_start(out=wft[:], in_=wf_d[h])
            wf_sb.append(wft)
        be_sb = const_p.tile([P, E], f32, tag="be", name="be_sb")
        nc.gpsimd.dma_start(out=be_sb[:], in_=be_d[:])

        # ---------------- resident tensors ----------------
        k_sb = []
        q_sb = []
        v_sb = []
        for h in range(2):
            k_sb.append(big_p.tile([P, T], f32r, tag=f"k{h}", name=f"k{h}"))
            q_sb.append(big_p.tile([P, T], f32r, tag=f"q{h}", name=f"q{h}"))
            v_sb.append(big_p.tile([P, T], f32r, tag=f"v{h}", name=f"v{h}"))
        vw_sb = big_p.tile([P, NST, EA], f8, tag="vw", name="vw_sb")
        nc.gpsimd.dma_start(
            out=vw_sb[:, :, E:EA], in_=oc_d[:].rearrange("p (n o) -> p n o", o=2)
        )

        # ---------------- phase 1: q, k, v -> vw' ----------------
        with (
            tc.tile_pool(name="ps_cv", bufs=4, space="PSUM") as ps_cv,
            tc.tile_pool(name="ps_vw", bufs=2, space="PSUM") as ps_vw_p,
        ):
            # dummy matmuls: ramp PE while wk/x DMAs land (results
            # discarded).  Grouped 4 per PSUM tile with a 1-column release
            # read -- too many unread accumulation groups stall the PE.
            scr_rd = const_p.tile([P, NDUMMY // 4], f32, tag="scrr", name="scr_rd")
            for b in range(NDUMMY // 4):
                ps_scr = ps_cv.tile([P, TCH], f32, tag="cv", name="ps_scr")
                for _ in range(4):
                    nc.tensor.matmul(
                        ps_scr[:], scr_w[:], scr_x[:], start=True, stop=True
                    )
                nc.vector.tensor_copy(scr_rd[:, b : b + 1], ps_scr[:, 0:1])

            def conv_pass(pi, emit):
                for h in range(2):
                    for j in range(NCH):
                        ps = ps_cv.tile([P, TCH], f32, tag="cv", name="ps_cv")
                        for kk in range(KW):
                            nc.tensor.matmul(
                                ps[:],
                                w_slice(pi, h, kk),
                                x_sb[:, XH0 + h * TP + j * TCH + kk : XH0 + h * TP + j * TCH + kk + TCH],
                                start=(kk == 0),
                                stop=(kk == KW - 1),
                            )
                        emit(h, j, ps)

            tsl_of = lambda j: slice(j * TCH, (j + 1) * TCH)
            conv_pass(
                1,
                lambda h, j, ps: nc.vector.tensor_scalar_add(
                    k_sb[h][:, tsl_of(j)], ps[:], bkq_sb[:, h : h + 1]
                ),
            )
            conv_pass(
                2, lambda h, j, ps: nc.vector.tensor_copy(v_sb[h][:, tsl_of(j)], ps[:])
            )

            # q-pass with vw matmuls interleaved at tap granularity: each
            # 512-col q-conv matmul hides the next vw LDWEIGHTS, so the
            # short (256-col) vw matmuls run back-to-back at full rate
            for h in range(2):
                for j in range(NCH):
                    jh = h * NCH + j
                    ps = ps_cv.tile([P, TCH], f32, tag="cv", name="ps_cv")
                    ps_v = ps_vw_p.tile([P, 2, E], f32, tag="vw", name="ps_vw")
                    vw_mms = []
                    for u in range(2):
                        tsl = slice((2 * jh + u) * P, (2 * jh + u + 1) * P)
                        vw_mms.append((ps_v[:, u, :], v_sb[0][:, tsl], wf_sb[0], True, False))
                        vw_mms.append((ps_v[:, u, :], v_sb[1][:, tsl], wf_sb[1], False, True))
                    for kk in range(KW):
                        nc.tensor.matmul(
                            ps[:],
                            w_slice(0, h, kk),
                            x_sb[:, XH0 + h * TP + j * TCH + kk : XH0 + h * TP + j * TCH + kk + TCH],
                            start=(kk == 0),
                            stop=(kk == KW - 1),
                        )
                        o, l, r, st, sp = vw_mms[kk]
                        nc.tensor.matmul(o, l[:], r[:], start=st, stop=sp)
                    o, l, r, st, sp = vw_mms[3]
                    nc.tensor.matmul(o, l[:], r[:], start=st, stop=sp)
                    nc.vector.tensor_scalar_add(
                        q_sb[h][:, tsl_of(j)], ps[:], bkq_sb[:, 2 + h : 3 + h]
                    )
                    nc.scalar.activation(
                        vw_sb[:, 2 * jh : 2 * jh + 2, 0:E], ps_v[:], AF.Copy
                    )

        # ---------------- phase 2: attention ----------------
        with (
            tc.tile_pool(name="ps_st", bufs=2, space="PSUM") as ps_st,
            tc.tile_pool(name="ps_u", bufs=1, space="PSUM") as ps_u,
        ):
            def st_pair(j, p):
                """S^T matmuls + one wide exp for si = 2p, 2p+1 vs chunk j."""
                ps = ps_st.tile([P, 2, TCH], f32, tag="st", name="ps_st")
                pt = pt_p.tile([P, 2, TCH], f8, tag="pt", name="pt")
                csl = slice(j * TCH, (j + 1) * TCH)
                for d in range(2):
                    ssl = slice((2 * p + d) * P, (2 * p + d + 1) * P)
                    nc.tensor.matmul(
                        ps[:, d, :], k_sb[0][:, ssl], q_sb[0][:, csl],
                        start=True, stop=False,
                    )
                    nc.tensor.matmul(
                        ps[:, d, :], k_sb[1][:, ssl], q_sb[1][:, csl],
                        start=False, stop=True,
                    )
                nc.scalar.activation(pt[:], ps[:], AF.Exp, bias=sh_sb[:])
                return pt

            def u_pair(p, pt, ups):
                """fp8 DoubleRow attn@v for si pair (2p, 2p+1): one matmul
                per t-subtile contracts both s-tiles (K=256) at 2x rate."""
                for ti in range(NSUB):
                    nc.tensor.matmul(
                        ups[ti][:],
                        pt[:, :, ti * P : (ti + 1) * P],
                        vw_sb[:, 2 * p : 2 * p + 2, :],
                        start=(p == 0),
                        stop=(p == NPAIR - 1),
                        perf_mode=DR,
                    )

            def drain(j, ups):
                """normalize + bias + store chunk j's four t-subtiles;
                y-store DMA issues alternate sync/scalar queues so the final
                chunk's stores overlap their own issue latency."""
                for ti in range(NSUB):
                    t0 = j * TCH + ti * P
                    rec = out_p.tile([P, 1], f32, tag="rec", name="rec")
                    nc.vector.reciprocal(rec[:], ups[ti][:, E : E + 1])
                    yt = out_p.tile([P, E], f32, tag="yt", name="yt")
                    nc.vector.scalar_tensor_tensor(
                        yt[:],
                        ups[ti][:, 0:E],
                        rec[:],
                        be_sb[:],
                        op0=mybir.AluOpType.mult,
                        op1=mybir.AluOpType.add,
                    )
                    eng = nc.scalar if (j == NCH - 1 and ti % 2 == 1) else nc.sync
                    eng.dma_start(out=y_d[t0 : t0 + P, :], in_=yt[:])

            # flat software pipeline with lag 2 ACROSS chunk boundaries:
            # attn@v for slot i runs after S^T of slot i+2, so each exp has a
            # full extra slot to finish, and the PE never drains between
            # chunks (chunk j+1's S^T matmuls overlap chunk j's tail attn@v
            # and its vector drain).
            LAG = 4
            slots = [(j, p) for j in range(NCH) for p in range(NPAIR)]
            ups_by_j = {}
            pts = {}

            def consume(i):
                j2, p2 = slots[i]
                if p2 == 0:
                    # allocate chunk j2's accumulators at first use; the
                    # prior chunk with these tags has fully drained by now
                    ups_by_j[j2] = [
                        ps_u.tile([P, EA], f32, tag=f"u{ti}", name=f"ups{ti}")
                        for ti in range(NSUB)
                    ]
                u_pair(p2, pts.pop(i), ups_by_j[j2])
                if p2 == NPAIR - 1:
                    drain(j2, ups_by_j.pop(j2))

            for i, (j, p) in enumerate(slots):
                pts[i] = st_pair(j, p)
                if i >= LAG:
                    consume(i - LAG)
            for i in range(len(slots) - LAG, len(slots)):
                consume(i)


def build_module():
    """Build + compile the Bass module (cached)."""
    global _MODULE
    if _MODULE is not None:
        return _MODULE
    nc = bacc.Bacc(
        "TRN2",
        target_bir_lowering=False,
        debug=False,
        enable_asserts=False,
        num_devices=NCORES,
    )
    f32 = dt.float32
    f32r = dt.float32r
    bf16 = dt.bfloat16
    x_d = nc.dram_tensor("x", [P, XCOLS], bf16, kind="ExternalInput").ap()
    wf_d = nc.dram_tensor("wfcT", [2, P, E], f32r, kind="ExternalInput").ap()
    be_d = nc.dram_tensor("beff", [P, E], f32, kind="ExternalInput").ap()
    oc_d = nc.dram_tensor("onescol", [P, NST * 2], dt.float8e4, kind="ExternalInput").ap()
    y_d = nc.dram_tensor("y", [T, E], f32, kind="ExternalOutput").ap()

    with tile.TileContext(nc) as tc:
        _build(tc, (x_d, wf_d, be_d, oc_d, y_d))
    nc.compile()
    _MODULE = nc
    return nc


def _marshal(wq, bq, wk, bk, wv, bv, w_fc, b_fc):
    """Host-side input prep (weights only -- all tiny)."""
    scale = np.float32(1.0 / np.sqrt(E))

    def blockdiag(w):
        # w: [E, E//H, KW] grouped conv weight ->
        # out[h, in_local, kk, out_local] block-diagonal per half.
        out = np.zeros((2, P, KW, P), np.float32)
        gs = E // H  # 32
        for h in range(2):
            for g in range(4):
                grp = 4 * h + g
                blk = w[gs * grp : gs * (grp + 1), :, :]  # [out c', in i, kk]
                for kk in range(KW):
                    out[h, gs * g : gs * (g + 1), kk, gs * g : gs * (g + 1)] = blk[
                        :, :, kk
                    ].T
        return out

    wqb = blockdiag(wq) * scale
    wkb = blockdiag(wk)
    wvb = blockdiag(wv)
    bq2 = np.ascontiguousarray((bq * scale).reshape(2, P).T)
    bk2 = np.ascontiguousarray(bk.reshape(2, P).T)
    wfcT = np.ascontiguousarray(w_fc.T.reshape(2, P, E))
    beff = np.ascontiguousarray(
        np.broadcast_to((w_fc @ bv + b_fc).reshape(1, E), (P, E))
    )
    bf = ml_dtypes.bfloat16
    # weight packs as [P, 768] bf16 rows: (h, kk, out) flattened per in-chan
    def wpack(w):
        return np.ascontiguousarray(
            w.transpose(1, 0, 2, 3).reshape(P, WCOLS).astype(bf)
        )
    return {
        "wkp": wpack(wkb),
        "wvp": wpack(wvb),
        "wqp": wpack(wqb),
        "bq2": bq2.astype(bf),
        "bk2": bk2.astype(bf),
        "wfcT": wfcT,
        "beff": beff,
        "onescol": np.ones((P, NST * 2), ml_dtypes.float8_e4m3),
    }


def kernel(x, wq, bq, wk, bk, wv, bv, w_fc, b_fc, num_heads):
    x = np.asarray(x, np.float32)
    consts = _marshal(
        np.asarray(wq, np.float32),
        np.asarray(bq, np.float32),
        np.asarray(wk, np.float32),
        np.asarray(bk, np.float32),
        np.asarray(wv, np.float32),
        np.asarray(bv, np.float32),
        np.asarray(w_fc, np.float32),
        np.asarray(b_fc, np.float32),
    )
    nc = build_module()
    # per-core flat x row: [h0 | h1 | wk | bk | bq | wv | wq] in bf16,
    # zero halo cols at both ends of each half
    xp = np.zeros((B, P, XCOLS), ml_dtypes.bfloat16)
    xb = x.astype(ml_dtypes.bfloat16)
    for b in range(B):
        for h in range(2):
            c0 = XH0 + h * TP + 1
            xp[b, :, c0 : c0 + T] = xb[b, h * P : (h + 1) * P, :]
        xp[b, :, XWK:XBK] = consts["wkp"]
        xp[b, :, XBK : XBK + 2] = consts["bk2"]
        xp[b, :, XBQ : XBQ + 2] = consts["bq2"]
        xp[b, :, XWV:XWQ] = consts["wvp"]
        xp[b, :, XWQ:XCOLS] = consts["wqp"]
    drop = ("wkp", "wvp", "wqp", "bq2", "bk2")
    wconsts = {k: v for k, v in consts.items() if k not in drop}
    in_maps = [{"x": np.ascontiguousarray(xp[b]), **wconsts} for b in range(B)]
    res = run_bass_kernel_spmd(nc, in_maps, core_ids=list(range(NCORES)), trace=TRACE)
    LAST["exec_time_ns"] = res.exec_time_ns
    LAST["mean_exec_time_ns"] = res.mean_exec_time_ns
    LAST["results"] = res
    out = np.stack([res.results[b]["y"] for b in range(B)], axis=0)
    return out


# revision 22
# speedup vs baseline: 1.0324x; 1.0031x over previous
"""ConvAttention kernel for 8x TRN2 NeuronCores.

Model (per batch item b):
    q/k/v = grouped_conv1d(x_b, w, b, groups=8)        # [E, T] -> [E, T]
    S     = (q^T k) / sqrt(E)                          # [T, T]
    P     = softmax(S, axis=-1)
    y     = (P @ v^T) @ w_fc^T + b_fc                  # [T, E]

Sharding: pure data-parallel over batch B=8 -> 8 cores, weights replicated.

Per-core algorithm (no transposes, scores never leave the chip):
  * x lives fully resident in SBUF as bf16 (host-padded halo); conv
    projections as block-diagonal [128,128] bf16 matmuls per tap, output in
    "ET" layout (channels on partitions) f32r -- what the scores matmul wants.
  * phase-1 structure: PE-warmup dummies (p-state ramp) -> k-pass -> v-pass
    -> q-pass with vw matmuls interleaved -> attention.  Everything the
    attention needs (k, q, vw) is finished when it starts, so the PE never
    stalls at the phase boundary.
  * fc is pushed in front of attention by associativity:
        y = P_norm @ (v_c @ w_fc^T + 1*beff)   with beff = w_fc@bv + b_fc
    (v's conv bias bv commutes through the softmax-normalized P).
  * scores are computed TRANSPOSED (S^T tiles, lhsT=k-tile, rhs=q-chunk) so
    that after exp the tiles are directly the stationary operand of attn@v.
  * softmax without max-subtraction (scores ~ N(0,1), exp is safe in fp32);
    row sums come for free from a ones-column appended to vw -> normalization
    is a per-partition reciprocal+scale on the final [128, 256] tiles.
  * scores matmuls in fp32r (full PE speed at N>=256); q/k tiles are
    *produced* as float32r by the bias-add, as walrus requires.
  * attention inner loop: per si-pair, 4 S^T matmuls -> one N=1024 exp
    (shifted by -3, output fp8e4) -> 4 fp8 DoubleRow attn@v matmuls (each
    contracts the full si-pair, K=256, at 2 MACs/cell/cycle) accumulating
    into 4 per-t-subtile PSUM banks, software-pipelined (S^T of pair p+1
    before attn@v of pair p).
  * fp8 error budget (simulated): P,vw in e4m3 + bf16 x/conv ->
    rel err ~1.6e-2 < 2e-2; scores stay fp32r (q/k fp8 would push it over).
"""

import contextlib

import ml_dtypes
import numpy as np

import concourse.bacc as bacc
import concourse.mybir as mybir
import concourse.tile as tile
from concourse.bass_utils import run_bass_kernel_spmd

dt = mybir.dt
AF = mybir.ActivationFunctionType
DR = mybir.MatmulPerfMode.DoubleRow
EXP_SHIFT = 3.0  # softmax shift: keeps exp() in fp8e4 range (max ~e^3.2 << 240)

B, E, T, H, KW = 8, 256, 4096, 8, 3
NCORES = 8
P = 128                  # partitions / half of E
TCH = 512                # t-chunk width
NCH = T // TCH           # 8 chunks
NST = T // P             # 32 s-tiles
NSUB = TCH // P          # 4 t-subtiles per chunk
NPAIR = NST // 2         # 16 si-pairs
EA = E + 2               # vw width incl. ones column (padded even for fp32r)
TP = T + 2               # padded x width (halo)
WCOLS = 2 * KW * P       # one projection's weight pack: 768 bf16 columns
XWK = 0                  # flat-x column offsets: wk pack leads
XBK = XWK + WCOLS        # bk (2 bf16 cols)
XBQ = XBK + 2            # bq (2 bf16 cols)
XH0 = XBQ + 2            # x half 0 (halo-padded)
XH1 = XH0 + TP           # x half 1
XWV = XH1 + TP           # wv pack
XWQ = XWV + WCOLS        # wq pack
XCOLS = XWQ + WCOLS      # total flat-x row width
NDUMMY = 12              # PE warm-up matmuls while first DMAs land

TRACE = False
LAST = {}

_MODULE = None


def _build(tc, io):
    nc = tc.nc
    f32 = dt.float32
    f32r = dt.float32r
    bf16 = dt.bfloat16
    f8 = dt.float8e4
    x_d, wf_d, be_d, oc_d, y_d = io

    with contextlib.ExitStack() as ctx:
        const_p = ctx.enter_context(tc.tile_pool(name="const", bufs=1))
        big_p = ctx.enter_context(tc.tile_pool(name="big", bufs=1))
        pt_p = ctx.enter_context(tc.tile_pool(name="ptp", bufs=6))
        out_p = ctx.enter_context(tc.tile_pool(name="outp", bufs=4))

        # ---- PE warm-up: scratch memsets (first gpsimd work), dummy matmuls
        # ramp the Tensor engine p-state while the first DMAs are in flight.
        scr_w = const_p.tile([P, P], bf16, tag="scrw", name="scr_w")
        nc.gpsimd.memset(scr_w[:], 0)
        scr_x = const_p.tile([P, TCH], bf16, tag="scrx", name="scr_x")
        nc.gpsimd.memset(scr_x[:], 0)

        # ---- DMA plan.  Facts: HWDGE descriptors cap at ~8KB/row (bigger
        # rows split and halve throughput), the sync ring starts ~8.6us,
        # the scalar ring ~11.3us, the gpsimd software-DGE ring ~14us and
        # slow.  x is one flat row [h0 | h1 | wk | bk | bq | wv | wq]:
        # (1) wk+biases (1.5KB rows) lead the sync ring -> land ~9.6us,
        # (2) h0 follows in two column halves, streaming just ahead of the
        #     k-pass,
        # (3) h1 halves ride the scalar ring,
        # (4) wv|wq close the sync ring, landing before the v-pass.
        x_sb = big_p.tile([P, XCOLS], bf16, tag="x", name="x_sb")
        TH = TP // 2 + 1
        nc.sync.dma_start(out=x_sb[:, 0 : XH0 + TH], in_=x_d[:, 0 : XH0 + TH])
        nc.scalar.dma_start(out=x_sb[:, XH0 + TH : XH1], in_=x_d[:, XH0 + TH : XH1])
        nc.sync.dma_start(out=x_sb[:, XH1 : XH1 + TH], in_=x_d[:, XH1 : XH1 + TH])
        nc.scalar.dma_start(out=x_sb[:, XH1 + TH : XWV], in_=x_d[:, XH1 + TH : XWV])
        nc.sync.dma_start(out=x_sb[:, XWV:XCOLS], in_=x_d[:, XWV:XCOLS])

        def w_slice(pi, h, kk):
            base = {1: XWK, 2: XWV, 0: XWQ}[pi]
            c0 = base + (h * KW + kk) * P
            return x_sb[:, c0 : c0 + P]

        # biases ride packed in x as bf16; widen to f32 on-chip (DVE
        # tensor_scalar requires an f32 scalar operand)
        bkq_sb = const_p.tile([P, 4], f32, tag="bkq", name="bkq_sb")
        nc.vector.tensor_copy(bkq_sb[:], x_sb[:, XBK : XBK + 4])
        sh_sb = const_p.tile([P, 1], f32, tag="sh", name="shift_sb")
        nc.gpsimd.memset(sh_sb[:], -EXP_SHIFT)
        wf_sb = []
        for h in range(2):
            wft = const_p.tile([P, E], f32r, tag=f"wf{h}", name=f"wf{h}")
            nc.gpsimd.dma_start(out=wft[:], in_=wf_d[h])
            wf_sb.append(wft)
        be_sb = const_p.tile([P, E], f32, tag="be", name="be_sb")
        nc.gpsimd.dma_start(out=be_sb[:], in_=be_d[:])

        # ---------------- resident tensors ----------------
        k_sb = []
        q_sb = []
        v_sb = []
        for h in range(2):
            k_sb.append(big_p.tile([P, T], f32r, tag=f"k{h}", name=f"k{h}"))
            q_sb.append(big_p.tile([P, T], f32r, tag=f"q{h}", name=f"q{h}"))
            v_sb.append(big_p.tile([P, T], f32r, tag=f"v{h}", name=f"v{h}"))
        vw_sb = big_p.tile([P, NST, EA], f8, tag="vw", name="vw_sb")
        nc.gpsimd.dma_start(
            out=vw_sb[:, :, E:EA], in_=oc_d[:].rearrange("p (n o) -> p n o", o=2)
        )

        # ---------------- phase 1: q, k, v -> vw' ----------------
        with (
            tc.tile_pool(name="ps_cv", bufs=4, space="PSUM") as ps_cv,
            tc.tile_pool(name="ps_vw", bufs=2, space="PSUM") as ps_vw_p,
        ):
            # dummy matmuls: ramp PE while wk/x DMAs land (results
            # discarded).  Grouped 4 per PSUM tile with a 1-column release
            # read -- too many unread accumulation groups stall the PE.
            scr_rd = const_p.tile([P, NDUMMY // 4], f32, tag="scrr", name="scr_rd")
            for b in range(NDUMMY // 4):
                ps_scr = ps_cv.tile([P, TCH], f32, tag="cv", name="ps_scr")
                for _ in range(4):
                    nc.tensor.matmul(
                        ps_scr[:], scr_w[:], scr_x[:], start=True, stop=True
                    )
                nc.vector.tensor_copy(scr_rd[:, b : b + 1], ps_scr[:, 0:1])

            def conv_pass(pi, emit):
                for h in range(2):
                    for j in range(NCH):
                        ps = ps_cv.tile([P, TCH], f32, tag="cv", name="ps_cv")
                        for kk in range(KW):
                            nc.tensor.matmul(
                                ps[:],
                                w_slice(pi, h, kk),
                                x_sb[:, XH0 + h * TP + j * TCH + kk : XH0 + h * TP + j * TCH + kk + TCH],
                                start=(kk == 0),
                                stop=(kk == KW - 1),
                            )
                        emit(h, j, ps)

            tsl_of = lambda j: slice(j * TCH, (j + 1) * TCH)
            conv_pass(
                1,
                lambda h, j, ps: nc.vector.tensor_scalar_add(
                    k_sb[h][:, tsl_of(j)], ps[:], bkq_sb[:, h : h + 1]
                ),
            )
            conv_pass(
                2, lambda h, j, ps: nc.vector.tensor_copy(v_sb[h][:, tsl_of(j)], ps[:])
            )

            # q-pass with vw matmuls interleaved at tap granularity: each
            # 512-col q-conv matmul hides the next vw LDWEIGHTS, so the
            # short (256-col) vw matmuls run back-to-back at full rate
            for h in range(2):
                for j in range(NCH):
                    jh = h * NCH + j
                    ps = ps_cv.tile([P, TCH], f32, tag="cv", name="ps_cv")
                    ps_v = ps_vw_p.tile([P, 2, E], f32, tag="vw", name="ps_vw")
                    vw_mms = []
                    for u in range(2):
                        tsl = slice((2 * jh + u) * P, (2 * jh + u + 1) * P)
                        vw_mms.append((ps_v[:, u, :], v_sb[0][:, tsl], wf_sb[0], True, False))
                        vw_mms.append((ps_v[:, u, :], v_sb[1][:, tsl], wf_sb[1], False, True))
                    for kk in range(KW):
                        nc.tensor.matmul(
                            ps[:],
                            w_slice(0, h, kk),
                            x_sb[:, XH0 + h * TP + j * TCH + kk : XH0 + h * TP + j * TCH + kk + TCH],
                            start=(kk == 0),
                            stop=(kk == KW - 1),
                        )
                        o, l, r, st, sp = vw_mms[kk]
                        nc.tensor.matmul(o, l[:], r[:], start=st, stop=sp)
                    o, l, r, st, sp = vw_mms[3]
                    nc.tensor.matmul(o, l[:], r[:], start=st, stop=sp)
                    nc.vector.tensor_scalar_add(
                        q_sb[h][:, tsl_of(j)], ps[:], bkq_sb[:, 2 + h : 3 + h]
                    )
                    nc.scalar.activation(
                        vw_sb[:, 2 * jh : 2 * jh + 2, 0:E], ps_v[:], AF.Copy
                    )

        # ---------------- phase 2: attention ----------------
        with (
            tc.tile_pool(name="ps_st", bufs=2, space="PSUM") as ps_st,
            tc.tile_pool(name="ps_u", bufs=1, space="PSUM") as ps_u,
        ):
            def st_pair(j, p):
                """S^T matmuls + one wide exp for si = 2p, 2p+1 vs chunk j."""
                ps = ps_st.tile([P, 2, TCH], f32, tag="st", name="ps_st")
                pt = pt_p.tile([P, 2, TCH], f8, tag="pt", name="pt")
                csl = slice(j * TCH, (j + 1) * TCH)
                for d in range(2):
                    ssl = slice((2 * p + d) * P, (2 * p + d + 1) * P)
                    nc.tensor.matmul(
                        ps[:, d, :], k_sb[0][:, ssl], q_sb[0][:, csl],
                        start=True, stop=False,
                    )
                    nc.tensor.matmul(
                        ps[:, d, :], k_sb[1][:, ssl], q_sb[1][:, csl],
                        start=False, stop=True,
                    )
                nc.scalar.activation(pt[:], ps[:], AF.Exp, bias=sh_sb[:])
                return pt

            def u_pair(p, pt, ups):
                """fp8 DoubleRow attn@v for si pair (2p, 2p+1): one matmul
                per t-subtile contracts both s-tiles (K=256) at 2x rate."""
                for ti in range(NSUB):
                    nc.tensor.matmul(
                        ups[ti][:],
                        pt[:, :, ti * P : (ti + 1) * P],
                        vw_sb[:, 2 * p : 2 * p + 2, :],
                        start=(p == 0),
                        stop=(p == NPAIR - 1),
                        perf_mode=DR,
                    )

            def drain(j, ups):
                """normalize + bias + store chunk j's four t-subtiles;
                y-store DMA issues alternate sync/scalar queues so the final
                chunk's stores overlap their own issue latency."""
                for ti in range(NSUB):
                    t0 = j * TCH + ti * P
                    rec = out_p.tile([P, 1], f32, tag="rec", name="rec")
                    nc.vector.reciprocal(rec[:], ups[ti][:, E : E + 1])
                    yt = out_p.tile([P, E], f32, tag="yt", name="yt")
                    nc.vector.scalar_tensor_tensor(
                        yt[:],
                        ups[ti][:, 0:E],
                        rec[:],
                        be_sb[:],
                        op0=mybir.AluOpType.mult,
                        op1=mybir.AluOpType.add,
                    )
                    eng = nc.scalar if (j == NCH - 1 and ti % 2 == 1) else nc.sync
                    eng.dma_start(out=y_d[t0 : t0 + P, :], in_=yt[:])

            # flat software pipeline with lag 2 ACROSS chunk boundaries:
            # attn@v for slot i runs after S^T of slot i+2, so each exp has a
            # full extra slot to finish, and the PE never drains between
            # chunks (chunk j+1's S^T matmuls overlap chunk j's tail attn@v
            # and its vector drain).
            LAG = 4
            slots = [(j, p) for j in range(NCH) for p in range(NPAIR)]
            ups_by_j = {}
            pts = {}

            def consume(i):
                j2, p2 = slots[i]
                if p2 == 0:
                    # allocate chunk j2's accumulators at first use; the
                    # prior chunk with these tags has fully drained by now
                    ups_by_j[j2] = [
                        ps_u.tile([P, EA], f32, tag=f"u{ti}", name=f"ups{ti}")
                        for ti in range(NSUB)
                    ]
                u_pair(p2, pts.pop(i), ups_by_j[j2])
                if p2 == NPAIR - 1:
                    drain(j2, ups_by_j.pop(j2))

            for i, (j, p) in enumerate(slots):
                pts[i] = st_pair(j, p)
                if i >= LAG:
                    consume(i - LAG)
            for i in range(len(slots) - LAG, len(slots)):
                consume(i)


def build_module():
    """Build + compile the Bass module (cached)."""
    global _MODULE
    if _MODULE is not None:
        return _MODULE
    nc = bacc.Bacc(
        "TRN2",
        target_bir_lowering=False,
        debug=False,
        enable_asserts=False,
        num_devices=NCORES,
    )
    f32 = dt.float32
    f32r = dt.float32r
    bf16 = dt.bfloat16
    x_d = nc.dram_tensor("x", [P, XCOLS], bf16, kind="ExternalInput").ap()
    wf_d = nc.dram_tensor("wfcT", [2, P, E], f32r, kind="ExternalInput").ap()
    be_d = nc.dram_tensor("beff", [P, E], f32, kind="ExternalInput").ap()
    oc_d = nc.dram_tensor("onescol", [P, NST * 2], dt.float8e4, kind="ExternalInput").ap()
    y_d = nc.dram_tensor("y", [T, E], f32, kind="ExternalOutput").ap()

    with tile.TileContext(nc) as tc:
        _build(tc, (x_d, wf_d, be_d, oc_d, y_d))
    nc.compile()
    _MODULE = nc
    return nc


def _marshal(wq, bq, wk, bk, wv, bv, w_fc, b_fc):
    """Host-side input prep (weights only -- all tiny)."""
    scale = np.float32(1.0 / np.sqrt(E))

    def blockdiag(w):
        # w: [E, E//H, KW] grouped conv weight ->
        # out[h, in_local, kk, out_local] block-diagonal per half.
        out = np.zeros((2, P, KW, P), np.float32)
        gs = E // H  # 32
        for h in range(2):
            for g in range(4):
                grp = 4 * h + g
                blk = w[gs * grp : gs * (grp + 1), :, :]  # [out c', in i, kk]
                for kk in range(KW):
                    out[h, gs * g : gs * (g + 1), kk, gs * g : gs * (g + 1)] = blk[
                        :, :, kk
                    ].T
        return out

    wqb = blockdiag(wq) * scale
    wkb = blockdiag(wk)
    wvb = blockdiag(wv)
    bq2 = np.ascontiguousarray((bq * scale).reshape(2, P).T)
    bk2 = np.ascontiguousarray(bk.reshape(2, P).T)
    wfcT = np.ascontiguousarray(w_fc.T.reshape(2, P, E))
    beff = np.ascontiguousarray(
        np.broadcast_to((w_fc @ bv + b_fc).reshape(1, E), (P, E))
    )
    bf = ml_dtypes.bfloat16
    # weight packs as [P, 768] bf16 rows: (h, kk, out) flattened per in-chan
    def wpack(w):
        return np.ascontiguousarray(
            w.transpose(1, 0, 2, 3).reshape(P, WCOLS).astype(bf)
        )
    return {
        "wkp": wpack(wkb),
        "wvp": wpack(wvb),
        "wqp": wpack(wqb),
        "bq2": bq2.astype(bf),
        "bk2": bk2.astype(bf),
        "wfcT": wfcT,
        "beff": beff,
        "onescol": np.ones((P, NST * 2), ml_dtypes.float8_e4m3),
    }


def kernel(x, wq, bq, wk, bk, wv, bv, w_fc, b_fc, num_heads):
    x = np.asarray(x, np.float32)
    consts = _marshal(
        np.asarray(wq, np.float32),
        np.asarray(bq, np.float32),
        np.asarray(wk, np.float32),
        np.asarray(bk, np.float32),
        np.asarray(wv, np.float32),
        np.asarray(bv, np.float32),
        np.asarray(w_fc, np.float32),
        np.asarray(b_fc, np.float32),
    )
    nc = build_module()
    # per-core flat x row: [h0 | h1 | wk | bk | bq | wv | wq] in bf16,
    # zero halo cols at both ends of each half
    xp = np.zeros((B, P, XCOLS), ml_dtypes.bfloat16)
    xb = x.astype(ml_dtypes.bfloat16)
    for b in range(B):
        for h in range(2):
            c0 = XH0 + h * TP + 1
            xp[b, :, c0 : c0 + T] = xb[b, h * P : (h + 1) * P, :]
        xp[b, :, XWK:XBK] = consts["wkp"]
        xp[b, :, XBK : XBK + 2] = consts["bk2"]
        xp[b, :, XBQ : XBQ + 2] = consts["bq2"]
        xp[b, :, XWV:XWQ] = consts["wvp"]
        xp[b, :, XWQ:XCOLS] = consts["wqp"]
    drop = ("wkp", "wvp", "wqp", "bq2", "bk2")
    wconsts = {k: v for k, v in consts.items() if k not in drop}
    in_maps = [{"x": np.ascontiguousarray(xp[b]), **wconsts} for b in range(B)]
    res = run_bass_kernel_spmd(nc, in_maps, core_ids=list(range(NCORES)), trace=TRACE)
    LAST["exec_time_ns"] = res.exec_time_ns
    LAST["mean_exec_time_ns"] = res.mean_exec_time_ns
    LAST["results"] = res
    out = np.stack([res.results[b]["y"] for b in range(B)], axis=0)
    return out


# revision 23
# speedup vs baseline: 1.0356x; 1.0032x over previous
"""ConvAttention kernel for 8x TRN2 NeuronCores.

Model (per batch item b):
    q/k/v = grouped_conv1d(x_b, w, b, groups=8)        # [E, T] -> [E, T]
    S     = (q^T k) / sqrt(E)                          # [T, T]
    P     = softmax(S, axis=-1)
    y     = (P @ v^T) @ w_fc^T + b_fc                  # [T, E]

Sharding: pure data-parallel over batch B=8 -> 8 cores, weights replicated.

Per-core algorithm (no transposes, scores never leave the chip):
  * x + conv weights + conv biases ship as ONE flat bf16 dram row per
    partition [wk | bk | bq | h0 | h1 | wv | wq] (halo pre-padded on host).
    Rationale (measured): HWDGE descriptors cap at ~8KB/row; the sync ring
    starts ~8.6us, the scalar ring ~11.3us, and the software-DGE (gpsimd)
    ring ~14us and slow -- so everything the k-pass gates on rides at the
    FRONT of the sync ring in one contiguous transfer, and the conv reads
    weights/biases as plain column slices of the resident x tile.
  * PE-warmup dummy matmuls (grouped 4 per PSUM tile with a 1-col release
    read; >8 unread accumulation groups stall the PE) ramp the Tensor
    engine p-state during the DMA wait, so the k-pass starts at full clock.
    Continuity is critical: any >~1us PE gap resets the clock to ~2x-slow
    for the next 3us.
  * phase 1: k-pass -> v-pass -> q-pass with the vw matmuls interleaved at
    tap granularity (each 512-col q-conv matmul hides the next 256-col vw
    matmul's LDWEIGHTS).  Conv = block-diagonal [128,128] bf16 matmuls per
    tap; q/k produced as f32r by the bias-add (walrus requirement).
  * fc is pushed in front of attention by associativity:
        y = P_norm @ (v_c @ w_fc^T + 1*beff)   with beff = w_fc@bv + b_fc
    (v's conv bias bv commutes through the softmax-normalized P).
  * scores are computed TRANSPOSED (S^T tiles, lhsT=k-tile, rhs=q-chunk) so
    that after exp the tiles are directly the stationary operand of attn@v.
  * softmax without max-subtraction (scores ~ N(0,1), exp is safe in fp32);
    row sums come for free from a ones-column appended to vw -> normalization
    is a per-partition reciprocal+scale on the final [128, 256] tiles.
  * attention inner loop: per si-pair slot, 4 f32r S^T matmuls -> one
    N=1024 exp (shifted by -3, output fp8e4) -> 4 fp8 DoubleRow attn@v
    matmuls (each contracts the full si-pair, K=256, at 2 MACs/cell/cycle)
    accumulating into 4 per-t-subtile PSUM banks.  The software pipeline is
    FLAT across chunk boundaries with lag 4, so chunk j+1's S^T matmuls
    cover chunk j's tail attn@v and its vector drain; slots run at the PE
    roofline (~1366ns = 4x512 + 4x258 cycles + one sem).
  * fp8 error budget (simulated + verified on HW): P,vw in e4m3 + bf16
    x/conv -> rel err ~1.64e-2 < 2e-2 gate; scores must stay f32r (q/k in
    fp8e4 measures 2.7e-2) and attn@v must stay fp8-DR (bf16 P/vw loses
    the 2x rate).  f32r and bf16 matmuls both run 1 col/cycle; fp8+DR is
    the only 2x mode, so the scores matmul is compute-bound at the f32r
    roofline (~176us of the ~236us total).
"""

import contextlib

import ml_dtypes
import numpy as np

import concourse.bacc as bacc
import concourse.mybir as mybir
import concourse.tile as tile
from concourse.bass_utils import run_bass_kernel_spmd

dt = mybir.dt
AF = mybir.ActivationFunctionType
DR = mybir.MatmulPerfMode.DoubleRow
EXP_SHIFT = 3.0  # softmax shift: keeps exp() in fp8e4 range (max ~e^3.2 << 240)

B, E, T, H, KW = 8, 256, 4096, 8, 3
NCORES = 8
P = 128                  # partitions / half of E
TCH = 512                # t-chunk width
NCH = T // TCH           # 8 chunks
NST = T // P             # 32 s-tiles
NSUB = TCH // P          # 4 t-subtiles per chunk
NPAIR = NST // 2         # 16 si-pairs
EA = E + 2               # vw width incl. ones column (padded even for fp32r)
TP = T + 2               # padded x width (halo)
WCOLS = 2 * KW * P       # one projection's weight pack: 768 bf16 columns
XWK = 0                  # flat-x column offsets: wk pack leads
XBK = XWK + WCOLS        # bk (2 bf16 cols)
XBQ = XBK + 2            # bq (2 bf16 cols)
XH0 = XBQ + 2            # x half 0 (halo-padded)
XH1 = XH0 + TP           # x half 1
XWV = XH1 + TP           # wv pack
XWQ = XWV + WCOLS        # wq pack
XCOLS = XWQ + WCOLS      # total flat-x row width
NDUMMY = 12              # PE warm-up matmuls while first DMAs land

TRACE = False
LAST = {}

_MODULE = None


def _build(tc, io):
    nc = tc.nc
    f32 = dt.float32
    f32r = dt.float32r
    bf16 = dt.bfloat16
    f8 = dt.float8e4
    x_d, wf_d, be_d, oc_d, y_d = io

    with contextlib.ExitStack() as ctx:
        const_p = ctx.enter_context(tc.tile_pool(name="const", bufs=1))
        big_p = ctx.enter_context(tc.tile_pool(name="big", bufs=1))
        pt_p = ctx.enter_context(tc.tile_pool(name="ptp", bufs=6))
        out_p = ctx.enter_context(tc.tile_pool(name="outp", bufs=4))

        # ---- PE warm-up: scratch memsets (first gpsimd work), dummy matmuls
        # ramp the Tensor engine p-state while the first DMAs are in flight.
        scr_w = const_p.tile([P, P], bf16, tag="scrw", name="scr_w")
        nc.gpsimd.memset(scr_w[:], 0)
        scr_x = const_p.tile([P, TCH], bf16, tag="scrx", name="scr_x")
        nc.gpsimd.memset(scr_x[:], 0)

        # ---- DMA plan.  Facts: HWDGE descriptors cap at ~8KB/row (bigger
        # rows split and halve throughput), the sync ring starts ~8.6us,
        # the scalar ring ~11.3us, the gpsimd software-DGE ring ~14us and
        # slow.  x is one flat row [h0 | h1 | wk | bk | bq | wv | wq]:
        # (1) wk+biases (1.5KB rows) lead the sync ring -> land ~9.6us,
        # (2) h0 follows in two column halves, streaming just ahead of the
        #     k-pass,
        # (3) h1 halves ride the scalar ring,
        # (4) wv|wq close the sync ring, landing before the v-pass.
        x_sb = big_p.tile([P, XCOLS], bf16, tag="x", name="x_sb")
        TH = TP // 2 + 1
        nc.sync.dma_start(out=x_sb[:, 0 : XH0 + TH], in_=x_d[:, 0 : XH0 + TH])
        nc.scalar.dma_start(out=x_sb[:, XH0 + TH : XH1], in_=x_d[:, XH0 + TH : XH1])
        nc.sync.dma_start(out=x_sb[:, XH1 : XH1 + TH], in_=x_d[:, XH1 : XH1 + TH])
        nc.scalar.dma_start(out=x_sb[:, XH1 + TH : XWV], in_=x_d[:, XH1 + TH : XWV])
        nc.sync.dma_start(out=x_sb[:, XWV:XCOLS], in_=x_d[:, XWV:XCOLS])

        def w_slice(pi, h, kk):
            base = {1: XWK, 2: XWV, 0: XWQ}[pi]
            c0 = base + (h * KW + kk) * P
            return x_sb[:, c0 : c0 + P]

        # biases ride packed in x as bf16; widen to f32 on-chip (DVE
        # tensor_scalar requires an f32 scalar operand)
        bkq_sb = const_p.tile([P, 4], f32, tag="bkq", name="bkq_sb")
        nc.vector.tensor_copy(bkq_sb[:], x_sb[:, XBK : XBK + 4])
        sh_sb = const_p.tile([P, 1], f32, tag="sh", name="shift_sb")
        nc.gpsimd.memset(sh_sb[:], -EXP_SHIFT)
        wf_sb = []
        for h in range(2):
            wft = const_p.tile([P, E], f32r, tag=f"wf{h}", name=f"wf{h}")
            nc.gpsimd.dma_start(out=wft[:], in_=wf_d[h])
            wf_sb.append(wft)
        be_sb = const_p.tile([P, E], f32, tag="be", name="be_sb")
        nc.gpsimd.dma_start(out=be_sb[:], in_=be_d[:])

        # ---------------- resident tensors ----------------
        k_sb = []
        q_sb = []
        v_sb = []
        for h in range(2):
            k_sb.append(big_p.tile([P, T], f32r, tag=f"k{h}", name=f"k{h}"))
            q_sb.append(big_p.tile([P, T], f32r, tag=f"q{h}", name=f"q{h}"))
            v_sb.append(big_p.tile([P, T], f32r, tag=f"v{h}", name=f"v{h}"))
        vw_sb = big_p.tile([P, NST, EA], f8, tag="vw", name="vw_sb")
        nc.gpsimd.dma_start(
            out=vw_sb[:, :, E:EA], in_=oc_d[:].rearrange("p (n o) -> p n o", o=2)
        )

        # ---------------- phase 1: q, k, v -> vw' ----------------
        with (
            tc.tile_pool(name="ps_cv", bufs=4, space="PSUM") as ps_cv,
            tc.tile_pool(name="ps_vw", bufs=2, space="PSUM") as ps_vw_p,
        ):
            # dummy matmuls: ramp PE while wk/x DMAs land (results
            # discarded).  Grouped 4 per PSUM tile with a 1-column release
            # read -- too many unread accumulation groups stall the PE.
            scr_rd = const_p.tile([P, NDUMMY // 4], f32, tag="scrr", name="scr_rd")
            for b in range(NDUMMY // 4):
                ps_scr = ps_cv.tile([P, TCH], f32, tag="cv", name="ps_scr")
                for _ in range(4):
                    nc.tensor.matmul(
                        ps_scr[:], scr_w[:], scr_x[:], start=True, stop=True
                    )
                nc.vector.tensor_copy(scr_rd[:, b : b + 1], ps_scr[:, 0:1])

            def conv_pass(pi, emit):
                for h in range(2):
                    for j in range(NCH):
                        ps = ps_cv.tile([P, TCH], f32, tag="cv", name="ps_cv")
                        for kk in range(KW):
                            nc.tensor.matmul(
                                ps[:],
                                w_slice(pi, h, kk),
                                x_sb[:, XH0 + h * TP + j * TCH + kk : XH0 + h * TP + j * TCH + kk + TCH],
                                start=(kk == 0),
                                stop=(kk == KW - 1),
                            )
                        emit(h, j, ps)

            tsl_of = lambda j: slice(j * TCH, (j + 1) * TCH)
            conv_pass(
                1,
                lambda h, j, ps: nc.vector.tensor_scalar_add(
                    k_sb[h][:, tsl_of(j)], ps[:], bkq_sb[:, h : h + 1]
                ),
            )
            conv_pass(
                2, lambda h, j, ps: nc.vector.tensor_copy(v_sb[h][:, tsl_of(j)], ps[:])
            )

            # q-pass with vw matmuls interleaved at tap granularity: each
            # 512-col q-conv matmul hides the next vw LDWEIGHTS, so the
            # short (256-col) vw matmuls run back-to-back at full rate
            for h in range(2):
                for j in range(NCH):
                    jh = h * NCH + j
                    ps = ps_cv.tile([P, TCH], f32, tag="cv", name="ps_cv")
                    ps_v = ps_vw_p.tile([P, 2, E], f32, tag="vw", name="ps_vw")
                    vw_mms = []
                    for u in range(2):
                        tsl = slice((2 * jh + u) * P, (2 * jh + u + 1) * P)
                        vw_mms.append((ps_v[:, u, :], v_sb[0][:, tsl], wf_sb[0], True, False))
                        vw_mms.append((ps_v[:, u, :], v_sb[1][:, tsl], wf_sb[1], False, True))
                    for kk in range(KW):
                        nc.tensor.matmul(
                            ps[:],
                            w_slice(0, h, kk),
                            x_sb[:, XH0 + h * TP + j * TCH + kk : XH0 + h * TP + j * TCH + kk + TCH],
                            start=(kk == 0),
                            stop=(kk == KW - 1),
                        )
                        o, l, r, st, sp = vw_mms[kk]
                        nc.tensor.matmul(o, l[:], r[:], start=st, stop=sp)
                    o, l, r, st, sp = vw_mms[3]
                    nc.tensor.matmul(o, l[:], r[:], start=st, stop=sp)
                    nc.vector.tensor_scalar_add(
                        q_sb[h][:, tsl_of(j)], ps[:], bkq_sb[:, 2 + h : 3 + h]
                    )
                    nc.scalar.activation(
                        vw_sb[:, 2 * jh : 2 * jh + 2, 0:E], ps_v[:], AF.Copy
                    )

        # ---------------- phase 2: attention ----------------
        with (
            tc.tile_pool(name="ps_st", bufs=2, space="PSUM") as ps_st,
            tc.tile_pool(name="ps_u", bufs=1, space="PSUM") as ps_u,
        ):
            def st_pair(j, p):
                """S^T matmuls + one wide exp for si = 2p, 2p+1 vs chunk j."""
                ps = ps_st.tile([P, 2, TCH], f32, tag="st", name="ps_st")
                pt = pt_p.tile([P, 2, TCH], f8, tag="pt", name="pt")
                csl = slice(j * TCH, (j + 1) * TCH)
                for d in range(2):
                    ssl = slice((2 * p + d) * P, (2 * p + d + 1) * P)
                    nc.tensor.matmul(
                        ps[:, d, :], k_sb[0][:, ssl], q_sb[0][:, csl],
                        start=True, stop=False,
                    )
                    nc.tensor.matmul(
                        ps[:, d, :], k_sb[1][:, ssl], q_sb[1][:, csl],
                        start=False, stop=True,
                    )
                nc.scalar.activation(pt[:], ps[:], AF.Exp, bias=sh_sb[:])
                return pt

            def u_pair(p, pt, ups):
                """fp8 DoubleRow attn@v for si pair (2p, 2p+1): one matmul
                per t-subtile contracts both s-tiles (K=256) at 2x rate."""
                for ti in range(NSUB):
                    nc.tensor.matmul(
                        ups[ti][:],
                        pt[:, :, ti * P : (ti + 1) * P],
                        vw_sb[:, 2 * p : 2 * p + 2, :],
                        start=(p == 0),
                        stop=(p == NPAIR - 1),
                        perf_mode=DR,
                    )

            def drain(j, ups):
                """normalize + bias + store chunk j's four t-subtiles;
                y-store DMA issues alternate sync/scalar queues so the final
                chunk's stores overlap their own issue latency."""
                for ti in range(NSUB):
                    t0 = j * TCH + ti * P
                    rec = out_p.tile([P, 1], f32, tag="rec", name="rec")
                    nc.vector.reciprocal(rec[:], ups[ti][:, E : E + 1])
                    yt = out_p.tile([P, E], f32, tag="yt", name="yt")
                    nc.vector.scalar_tensor_tensor(
                        yt[:],
                        ups[ti][:, 0:E],
                        rec[:],
                        be_sb[:],
                        op0=mybir.AluOpType.mult,
                        op1=mybir.AluOpType.add,
                    )
                    eng = nc.scalar if (j == NCH - 1 and ti % 2 == 1) else nc.sync
                    eng.dma_start(out=y_d[t0 : t0 + P, :], in_=yt[:])

            # flat software pipeline with lag 2 ACROSS chunk boundaries:
            # attn@v for slot i runs after S^T of slot i+2, so each exp has a
            # full extra slot to finish, and the PE never drains between
            # chunks (chunk j+1's S^T matmuls overlap chunk j's tail attn@v
            # and its vector drain).
            LAG = 4
            slots = [(j, p) for j in range(NCH) for p in range(NPAIR)]
            ups_by_j = {}
            pts = {}

            def consume(i):
                j2, p2 = slots[i]
                if p2 == 0:
                    # allocate chunk j2's accumulators at first use; the
                    # prior chunk with these tags has fully drained by now
                    ups_by_j[j2] = [
                        ps_u.tile([P, EA], f32, tag=f"u{ti}", name=f"ups{ti}")
                        for ti in range(NSUB)
                    ]
                u_pair(p2, pts.pop(i), ups_by_j[j2])
                if p2 == NPAIR - 1:
                    drain(j2, ups_by_j.pop(j2))

            for i, (j, p) in enumerate(slots):
                pts[i] = st_pair(j, p)
                if i >= LAG:
                    consume(i - LAG)
            for i in range(len(slots) - LAG, len(slots)):
                consume(i)


def build_module():
    """Build + compile the Bass module (cached)."""
    global _MODULE
    if _MODULE is not None:
        return _MODULE
    nc = bacc.Bacc(
        "TRN2",
        target_bir_lowering=False,
        debug=False,
        enable_asserts=False,
        num_devices=NCORES,
    )
    f32 = dt.float32
    f32r = dt.float32r
    bf16 = dt.bfloat16
    x_d = nc.dram_tensor("x", [P, XCOLS], bf16, kind="ExternalInput").ap()
    wf_d = nc.dram_tensor("wfcT", [2, P, E], f32r, kind="ExternalInput").ap()
    be_d = nc.dram_tensor("beff", [P, E], f32, kind="ExternalInput").ap()
    oc_d = nc.dram_tensor("onescol", [P, NST * 2], dt.float8e4, kind="ExternalInput").ap()
    y_d = nc.dram_tensor("y", [T, E], f32, kind="ExternalOutput").ap()

    with tile.TileContext(nc) as tc:
        _build(tc, (x_d, wf_d, be_d, oc_d, y_d))
    nc.compile()
    _MODULE = nc
    return nc


def _marshal(wq, bq, wk, bk, wv, bv, w_fc, b_fc):
    """Host-side input prep (weights only -- all tiny)."""
    scale = np.float32(1.0 / np.sqrt(E))

    def blockdiag(w):
        # w: [E, E//H, KW] grouped conv weight ->
        # out[h, in_local, kk, out_local] block-diagonal per half.
        out = np.zeros((2, P, KW, P), np.float32)
        gs = E // H  # 32
        for h in range(2):
            for g in range(4):
                grp = 4 * h + g
                blk = w[gs * grp : gs * (grp + 1), :, :]  # [out c', in i, kk]
                for kk in range(KW):
                    out[h, gs * g : gs * (g + 1), kk, gs * g : gs * (g + 1)] = blk[
                        :, :, kk
                    ].T
        return out

    wqb = blockdiag(wq) * scale
    wkb = blockdiag(wk)
    wvb = blockdiag(wv)
    bq2 = np.ascontiguousarray((bq * scale).reshape(2, P).T)
    bk2 = np.ascontiguousarray(bk.reshape(2, P).T)
    wfcT = np.ascontiguousarray(w_fc.T.reshape(2, P, E))
    beff = np.ascontiguousarray(
        np.broadcast_to((w_fc @ bv + b_fc).reshape(1, E), (P, E))
    )
    bf = ml_dtypes.bfloat16
    # weight packs as [P, 768] bf16 rows: (h, kk, out) flattened per in-chan
    def wpack(w):
        return np.ascontiguousarray(
            w.transpose(1, 0, 2, 3).reshape(P, WCOLS).astype(bf)
        )
    return {
        "wkp": wpack(wkb),
        "wvp": wpack(wvb),
        "wqp": wpack(wqb),
        "bq2": bq2.astype(bf),
        "bk2": bk2.astype(bf),
        "wfcT": wfcT,
        "beff": beff,
        "onescol": np.ones((P, NST * 2), ml_dtypes.float8_e4m3),
    }


def kernel(x, wq, bq, wk, bk, wv, bv, w_fc, b_fc, num_heads):
    x = np.asarray(x, np.float32)
    consts = _marshal(
        np.asarray(wq, np.float32),
        np.asarray(bq, np.float32),
        np.asarray(wk, np.float32),
        np.asarray(bk, np.float32),
        np.asarray(wv, np.float32),
        np.asarray(bv, np.float32),
        np.asarray(w_fc, np.float32),
        np.asarray(b_fc, np.float32),
    )
    nc = build_module()
    # per-core flat x row: [h0 | h1 | wk | bk | bq | wv | wq] in bf16,
    # zero halo cols at both ends of each half
    xp = np.zeros((B, P, XCOLS), ml_dtypes.bfloat16)
    xb = x.astype(ml_dtypes.bfloat16)
    for b in range(B):
        for h in range(2):
            c0 = XH0 + h * TP + 1
            xp[b, :, c0 : c0 + T] = xb[b, h * P : (h + 1) * P, :]
        xp[b, :, XWK:XBK] = consts["wkp"]
        xp[b, :, XBK : XBK + 2] = consts["bk2"]
        xp[b, :, XBQ : XBQ + 2] = consts["bq2"]
        xp[b, :, XWV:XWQ] = consts["wvp"]
        xp[b, :, XWQ:XCOLS] = consts["wqp"]
    drop = ("wkp", "wvp", "wqp", "bq2", "bk2")
    wconsts = {k: v for k, v in consts.items() if k not in drop}
    in_maps = [{"x": np.ascontiguousarray(xp[b]), **wconsts} for b in range(B)]
    res = run_bass_kernel_spmd(nc, in_maps, core_ids=list(range(NCORES)), trace=TRACE)
    LAST["exec_time_ns"] = res.exec_time_ns
    LAST["mean_exec_time_ns"] = res.mean_exec_time_ns
    LAST["results"] = res
    out = np.stack([res.results[b]["y"] for b in range(B)], axis=0)
    return out
